# revision 29
# baseline (speedup 1.0000x reference)
"""Trainium2 Bass kernel for nn_EntityClassify (2-layer R-GCN on 8 NeuronCores).

Math (matches reference):
  h1  = relu(bias1 + sum_r S_r @ embed)          S_r = right-normalized adjacency
  out = bias2 + sum_r S_r @ (h1 @ W_r)

Distribution: destination nodes sharded across 8 cores; embed + weights
replicated. Aggregation is race-free one-hot matmul accumulation in PSUM
(HW dma_scatter_add loses colliding updates across its 16 parallel engines,
so no scatter-add is used).

v2 structure (per 128-edge tile, one shared edge schedule for both layers,
sorted by (src chunk, dst block), padded per group to x128):

  L1: dma_gather embed rows (fp16) -> st [128e, H]; M = onehot(dst)*w via one
      fused tensor_scalar (is_equal + mult against an iota row); matmul
      lhsT=M rhs=st accumulates PSUM[128d, H] per (chunk, block); group end
      adds into acc1 [128d, nblk*H] (node-major - no transpose needed).
      Epilogue: h1 = relu(acc1 + b1row) -> fp16 -> DRAM.
  AllGather h1 (fp16).
  L2: transposed dma_gather of h1 -> mst [H, e]; xw matmul lhsT=mst_tile
      rhs=Wcat [H, 256] (o-major slots o*4+r) -> PSUM xp [128e, 256]; DVE
      mult xs = xp * wmask4 (w_e * onehot(rel), inner-4 broadcast) casts to
      fp16 and does rel-selection + norm in one pass; agg matmul lhsT=M
      rhs=xs -> PSUM [128d, 256]; group end folds 4 rel slots into acc2
      [128d, nblk*O]. Epilogue: out = acc2 + b2row.

Host precomputes the shared per-core edge schedule; per-(chunk,block)
capacities are maxed over cores so the single SPMD program fits every core;
pads use (idx=0, w=0, dl=999) no-op edges.
"""

import os
import sys

import numpy as np

sys.path.insert(0, "/opt/trn_rl_repo")

NCORES = 8
NCHUNKS = 4
BATCH = 2048  # max indices per dma_gather call (needs single_packet=False on HW)
P = 128

last_results = None  # BassKernelResults of the most recent hw run


def _round_up(x, m):
    return (x + m - 1) // m * m


def _wrap16(idx, n):
    """SWDGE index layout: position j -> [j%16, j//16]; 16 rows replicated x8."""
    a = idx.reshape(n // 16, 16).T.astype(np.int16)
    return np.tile(a, (8, 1))


def _build_schedule(src, dst, rel, wts, chunk, shard, nblk):
    """Sort edges by (chunk, block); pad per group to x128 with no-op edges.

    Returns per-core packed streams plus the shared group capacities:
      gidx  int16 chunk-local src index
      warr  fp16 edge weight (1/deg)
      dlarr fp16 dst local-within-block column (999 for pads)
      rarr  int8 relation id (0 for pads; pads have w=0)
    """
    core = dst // shard
    block = (dst % shard) // P
    group = (src // chunk) * nblk + block
    ngroups = NCHUNKS * nblk

    per_core = []
    counts = np.zeros((NCORES, ngroups), np.int64)
    for k in range(NCORES):
        m = core == k
        g = group[m]
        order = np.argsort(g, kind="stable")
        per_core.append((g[order], src[m][order], wts[m][order], rel[m][order],
                         dst[m][order]))
        counts[k] = np.bincount(g, minlength=ngroups)
    caps = _round_up(counts.max(axis=0), P)
    offs = np.concatenate([[0], np.cumsum(caps)])
    tot = int(caps.sum())

    gidx = np.zeros((NCORES, tot), np.int16)
    warr = np.zeros((NCORES, tot), np.float16)
    dlarr = np.full((NCORES, tot), 999.0, np.float16)  # pads never match iota
    rarr = np.zeros((NCORES, tot), np.int8)
    for k in range(NCORES):
        g, s, w, r, d = per_core[k]
        gs = np.searchsorted(g, np.arange(ngroups))
        ge = np.searchsorted(g, np.arange(ngroups), side="right")
        for gi in range(ngroups):
            a, b = gs[gi], ge[gi]
            o = offs[gi]
            n = b - a
            gidx[k, o : o + n] = (s[a:b] % chunk).astype(np.int16)
            warr[k, o : o + n] = w[a:b].astype(np.float16)
            dlarr[k, o : o + n] = ((d[a:b] % shard) % P).astype(np.float16)
            rarr[k, o : o + n] = r[a:b].astype(np.int8)
    return gidx, warr, dlarr, rarr, caps, offs


def _make_calls(caps, nblk):
    """Chunk-pure gather-call windows (<=BATCH, x128) over the packed stream,
    plus per-tile group bookkeeping: (group, is_first, is_last)."""
    offs = np.concatenate([[0], np.cumsum(caps)])
    calls = []  # (chunk, stream_off, n)
    for c in range(NCHUNKS):
        lo = int(offs[c * nblk])
        hi = int(offs[(c + 1) * nblk])
        o = lo
        while o < hi:
            n = min(BATCH, hi - o)
            calls.append((c, o, n))
            o += n
    ntiles_total = int(caps.sum()) // P
    tile_group = np.zeros(ntiles_total, np.int64)
    for gi, cap in enumerate(caps):
        t0 = int(offs[gi]) // P
        for t in range(int(cap) // P):
            tile_group[t0 + t] = gi
    tiles = []
    for t in range(ntiles_total):
        g = tile_group[t]
        first = t == 0 or tile_group[t - 1] != g
        last = t == ntiles_total - 1 or tile_group[t + 1] != g
        tiles.append((int(g), first, last))
    return calls, tiles


def _host_schedules(embed, weight, bias1, bias2, edge_src, edge_dst):
    N, H = embed.shape
    R, _, O = weight.shape
    E = edge_src.shape[1]
    shard = _round_up((N + NCORES - 1) // NCORES, P)
    npad = shard * NCORES
    chunk = npad // NCHUNKS
    nblk = shard // P
    assert chunk < 32768 and shard < 32768

    es = edge_src.astype(np.int64).reshape(R, E)
    ed = edge_dst.astype(np.int64).reshape(R, E)
    deg = np.zeros((R, N), np.float32)
    for r in range(R):
        deg[r] = np.bincount(ed[r], minlength=N)
    dinv = 1.0 / np.maximum(deg, 1.0)

    src = es.reshape(-1)
    dst = ed.reshape(-1)
    rel = np.repeat(np.arange(R), E)
    w = dinv[rel, dst]

    g, warr, dlarr, rarr, caps, _ = _build_schedule(
        src, dst, rel, w, chunk, shard, nblk
    )
    calls, tiles = _make_calls(caps, nblk)
    tot = int(caps.sum())
    ntiles = tot // P

    # wmask4[e, r'] = w_e if r' == rel_e else 0  (pads: all 0)
    wm4 = np.zeros((NCORES, tot, R), np.float16)
    for k in range(NCORES):
        wm4[k, np.arange(tot), rarr[k].astype(np.int64)] = warr[k]

    iota = np.zeros((P, P), np.float16)
    iota[:] = np.arange(P, dtype=np.float16)[None, :]
    # Wcat o-major: Wc[h, o*R + r] = W[r, h, o]
    wcat = np.ascontiguousarray(
        weight.astype(np.float16).transpose(1, 2, 0).reshape(H, O * R)
    )
    b1rep = np.tile(bias1.astype(np.float32)[None, :], (P, 1))
    b2rep = np.tile(bias2.astype(np.float32)[None, :], (P, 1))

    consts = dict(
        N=N, H=H, R=R, O=O, shard=shard, npad=npad, chunk=chunk, nblk=nblk,
        tot=tot, ntiles=ntiles, calls=calls, tiles=tiles,
    )
    embed_pad = np.zeros((npad, H), np.float16)
    embed_pad[:N] = embed.astype(np.float16)
    in_maps = []
    for k in range(NCORES):
        in_maps.append(
            dict(
                embed=embed_pad,
                wcat=wcat,
                b1rep=b1rep,
                b2rep=b2rep,
                iota=iota,
                gall=_wrap16(g[k], tot),
                wall=warr[k].reshape(ntiles, P).T.astype(np.float32).copy(),
                dall=dlarr[k].reshape(ntiles, P).T.astype(np.float32).copy(),
                wm4=np.ascontiguousarray(
                    wm4[k].reshape(ntiles, P, R).transpose(1, 0, 2)
                ),
            )
        )
    return consts, in_maps


def _simulate_numpy(consts, in_maps):
    """Numpy model of exactly what the device program computes."""
    shard, chunk, H, O, R, nblk = (
        consts["shard"], consts["chunk"], consts["H"], consts["O"],
        consts["R"], consts["nblk"],
    )
    tot = consts["tot"]

    def unwrap(a):
        return a[:16].T.reshape(-1)[:tot].astype(np.int64)

    h1_all = []
    for k in range(NCORES):
        m = in_maps[k]
        gi = unwrap(m["gall"])
        w = m["wall"].T.reshape(-1)
        dl = m["dall"].T.reshape(-1)
        acc = np.zeros((P, nblk, H), np.float32)
        for t, (g, first, last) in enumerate(consts["tiles"]):
            c, b = g // nblk, g % nblk
            sl = slice(t * P, (t + 1) * P)
            st = m["embed"][c * chunk + gi[sl]].astype(np.float32)  # [e, h]
            M = (dl[sl][:, None] == np.arange(P)).astype(np.float32)
            M *= w[sl].astype(np.float32)[:, None]
            acc[:, b, :] += M.T @ st
        h1 = np.maximum(
            acc.transpose(1, 0, 2).reshape(shard, H) + m["b1rep"][0], 0
        ).astype(np.float16)
        h1_all.append(h1)
    h1f = np.concatenate(h1_all, 0)

    outs = []
    for k in range(NCORES):
        m = in_maps[k]
        gi = unwrap(m["gall"])
        dl = m["dall"].T.reshape(-1)
        acc = np.zeros((P, nblk, O), np.float32)
        wc = m["wcat"].astype(np.float32)  # [H, O*R]
        for t, (g, first, last) in enumerate(consts["tiles"]):
            c, b = g // nblk, g % nblk
            sl = slice(t * P, (t + 1) * P)
            mst = h1f[c * chunk + gi[sl]].astype(np.float32)  # [e, h]
            xp = mst @ wc  # [e, O*R]
            wm = m["wm4"][:, t, :].astype(np.float32)  # [e, R]
            xs = (xp.reshape(P, O, R) * wm[:, None, :]).reshape(P, O * R)
            xs = xs.astype(np.float16).astype(np.float32)
            M = (dl[sl][:, None] == np.arange(P)).astype(np.float32)
            ps = M.T @ xs  # [d, O*R]
            acc[:, b, :] += ps.reshape(P, O, R).sum(2)
        outs.append(acc.transpose(1, 0, 2).reshape(shard, O) + m["b2rep"][0])
    return np.concatenate(outs, 0)[: consts["N"]]


def _build_program(consts, finalize, variant=(), kloop=1, nqueues=1,
                   single_packet=False):
    variant = frozenset(variant)
    import concourse.bacc as bacc
    import concourse.mybir as mybir
    import concourse.tile as tile
    from concourse import library_config

    f32 = mybir.dt.float32
    f16 = mybir.dt.float16
    i16 = mybir.dt.int16
    AF = mybir.ActivationFunctionType
    H, O, R = consts["H"], consts["O"], consts["R"]
    shard, npad, chunk, nblk = (
        consts["shard"], consts["npad"], consts["chunk"], consts["nblk"],
    )
    tot, ntiles = consts["tot"], consts["ntiles"]

    nc = bacc.Bacc("TRN2", num_swdge_queues=nqueues)
    embed = nc.declare_dram_parameter("embed", [npad, H], f16, isOutput=False)
    wcat = nc.declare_dram_parameter("wcat", [H, O * R], f16, isOutput=False)
    b1rep = nc.declare_dram_parameter("b1rep", [P, H], f32, isOutput=False)
    b2rep = nc.declare_dram_parameter("b2rep", [P, O], f32, isOutput=False)
    iota = nc.declare_dram_parameter("iota", [P, P], f16, isOutput=False)
    gall = nc.declare_dram_parameter("gall", [P, tot // 16], i16, isOutput=False)
    wall = nc.declare_dram_parameter("wall", [P, ntiles], f32, isOutput=False)
    dall = nc.declare_dram_parameter("dall", [P, ntiles], f32, isOutput=False)
    wm4 = nc.declare_dram_parameter("wm4", [P, ntiles, R], f16, isOutput=False)
    out = nc.declare_dram_parameter("out", [shard, O], f32, isOutput=True)

    h1l = nc.dram_tensor("h1l", [shard, H], f16)
    h1f = nc.dram_tensor("h1f", [npad, H], f16, addr_space="Shared")

    with tile.TileContext(nc) as tc:
        with (
            tc.tile_pool(name="const", bufs=1) as cpool,
            tc.tile_pool(name="acc", bufs=1) as apool,
            tc.tile_pool(name="stage", bufs=4) as spool,
            tc.tile_pool(name="mbuf", bufs=3) as mpool,
            tc.tile_pool(name="xs", bufs=4) as xpool,
            tc.tile_pool(name="ep", bufs=3) as epool,
            tc.tile_pool(name="agg1_ps", bufs=2, space="PSUM") as qagg1,
            tc.tile_pool(name="xw_ps", bufs=3, space="PSUM") as qxw,
            tc.tile_pool(name="agg2_ps", bufs=2, space="PSUM") as qagg2,
        ):
            nc.gpsimd.load_library(library_config.mlp)

            _regs = {}

            def nreg(n):
                if n not in _regs:
                    r = nc.gpsimd.alloc_register(name=f"nidx{n}")
                    nc.gpsimd.reg_mov(r, n)
                    _regs[n] = r
                return _regs[n]

            b1t = cpool.tile([P, H], f32)
            nc.sync.dma_start(out=b1t[:], in_=b1rep[:, :])
            b2t = cpool.tile([P, O], f32)
            nc.sync.dma_start(out=b2t[:], in_=b2rep[:, :])
            iot = cpool.tile([P, P], f16)
            nc.sync.dma_start(out=iot[:], in_=iota[:, :])
            wct = cpool.tile([H, O * R], f16)
            nc.sync.dma_start(out=wct[:], in_=wcat[:, :])
            # whole-stream index/weight tables resident in SBUF
            gt = cpool.tile([P, tot // 16], i16)
            nc.sync.dma_start(out=gt[:], in_=gall[:, :])
            wt = cpool.tile([P, ntiles], f32)
            nc.sync.dma_start(out=wt[:], in_=wall[:, :])
            dt = cpool.tile([P, ntiles], f32)
            nc.sync.dma_start(out=dt[:], in_=dall[:, :])
            wmt = cpool.tile([P, ntiles, R], f16)
            nc.sync.dma_start(out=wmt[:], in_=wm4[:, :, :])

            qb = (nblk + 3) // 4  # blocks per accumulator quarter

            for _rep in range(kloop):
                acc1 = []
                acc2 = []
                for q in range(4):
                    a1 = apool.tile([P, qb, H], f32, tag=f"acc1_{q}",
                                    name=f"acc1_{q}_{_rep}")
                    nc.any.memset(a1[:], 0.0)
                    acc1.append(a1)
                    a2 = apool.tile([P, qb, O], f32, tag=f"acc2_{q}",
                                    name=f"acc2_{q}_{_rep}")
                    nc.any.memset(a2[:], 0.0)
                    acc2.append(a2)

                # ---------------- layer 1 ----------------
                psums = {}
                t_base = 0
                for c, off, n in ([] if "no_l1" in variant else consts["calls"]):
                    k = n // P
                    st = spool.tile([P, BATCH // P, H], f16, tag="st1")
                    if "no_gather" not in variant:
                        nc.gpsimd.dma_gather(
                            out_ap=st[:, :k, :],
                            in_ap=embed[c * chunk : (c + 1) * chunk, :],
                            idxs_ap=gt[:, off // 16 : (off + n) // 16],
                            num_idxs=n,
                            num_idxs_reg=nreg(n),
                            elem_size=H,
                            single_packet=single_packet,
                            queue_num=(off // BATCH) % nqueues,
                        )
                    if "gather_only" in variant:
                        t_base += k
                        continue
                    mb = mpool.tile([P, BATCH // P, P], f16, tag="m1")
                    for t in range(k):
                        tt = t_base + t
                        g, first, last = consts["tiles"][tt]
                        nc.vector.tensor_scalar(
                            mb[:, t, :],
                            iot[:, :],
                            dt[:, tt : tt + 1],
                            wt[:, tt : tt + 1],
                            op0=mybir.AluOpType.is_equal,
                            op1=mybir.AluOpType.mult,
                        )
                        if first:
                            psums[g] = qagg1.tile(
                                [P, H], f32, tag="agg1", name=f"agg1_{g}_{_rep}"
                            )
                        nc.tensor.matmul(
                            psums[g][:],
                            lhsT=mb[:, t, :],
                            rhs=st[:, t, :],
                            start=first,
                            stop=last,
                        )
                        if last:
                            b = g % nblk
                            a1 = acc1[b // qb][:, b % qb, :]
                            nc.any.tensor_add(a1, a1, psums[g][:])
                            del psums[g]
                    t_base += k

                # ---- h1 = relu(acc1 + b1row) -> fp16, already node-major
                if "no_l1" not in variant:
                    for b in range(nblk):
                        hb = epool.tile([P, H], f32, tag="ep_h")
                        nc.any.tensor_add(
                            hb[:], acc1[b // qb][:, b % qb, :], b1t[:, :]
                        )
                        hf = epool.tile([P, H], f16, tag="ep_f")
                        nc.scalar.activation(hf[:], hb[:], AF.Relu)
                        nc.sync.dma_start(
                            out=h1l[b * P : (b + 1) * P, :], in_=hf[:]
                        )

                # ---- all-gather h1
                if "no_coll" not in variant:
                    nc.gpsimd.collective_compute(
                        "AllGather",
                        mybir.AluOpType.bypass,
                        replica_groups=[list(range(NCORES))],
                        ins=[h1l[:, :]],
                        outs=[h1f[:, :]],
                    )

                # ---------------- layer 2 ----------------
                psums = {}
                t_base = 0
                for c, off, n in ([] if "no_l2" in variant else consts["calls"]):
                    k = n // P
                    mst = spool.tile([P, 1, BATCH], f16, tag="st2")
                    if "no_gather" not in variant:
                        nc.gpsimd.dma_gather(
                            out_ap=mst[:, :, :n],
                            in_ap=h1f[c * chunk : (c + 1) * chunk, :],
                            idxs_ap=gt[:, off // 16 : (off + n) // 16],
                            num_idxs=n,
                            num_idxs_reg=nreg(n),
                            elem_size=H,
                            transpose=True,
                            single_packet=single_packet,
                            queue_num=(off // BATCH) % nqueues,
                        )
                    if "gather_only" in variant:
                        t_base += k
                        continue
                    mb = mpool.tile([P, BATCH // P, P], f16, tag="m2")
                    for t in range(k):
                        tt = t_base + t
                        g, first, last = consts["tiles"][tt]
                        xp = qxw.tile([P, O * R], f32, tag="xwp")
                        nc.tensor.matmul(
                            xp[:],
                            lhsT=mst[:, 0, t * P : (t + 1) * P],
                            rhs=wct[:, :],
                            start=True,
                            stop=True,
                        )
                        xs = xpool.tile([P, O, R], f16, tag="xss")
                        nc.vector.tensor_tensor(
                            xs[:, :, :],
                            xp[:].rearrange("p (o r) -> p o r", r=R),
                            wmt[:, tt, None, :].to_broadcast([P, O, R]),
                            op=mybir.AluOpType.mult,
                        )
                        nc.vector.tensor_scalar(
                            mb[:, t, :],
                            iot[:, :],
                            dt[:, tt : tt + 1],
                            None,
                            op0=mybir.AluOpType.is_equal,
                        )
                        if first:
                            psums[g] = qagg2.tile(
                                [P, O * R], f32, tag="agg2", name=f"agg2_{g}_{_rep}"
                            )
                        nc.tensor.matmul(
                            psums[g][:],
                            lhsT=mb[:, t, :],
                            rhs=xs[:, :, :].rearrange("p o r -> p (o r)"),
                            start=first,
                            stop=last,
                        )
                        if last:
                            b = g % nblk
                            a2 = acc2[b // qb][:, b % qb, :]
                            ps = psums[g][:].rearrange("p (o r) -> p o r", r=R)
                            for r in range(R):
                                nc.any.tensor_add(a2, a2, ps[:, :, r])
                            del psums[g]
                    t_base += k

                # ---- out = acc2 + b2row (node-major already)
                for b in range(nblk):
                    ob = epool.tile([P, O], f32, tag="ep_o")
                    nc.any.tensor_add(
                        ob[:], acc2[b // qb][:, b % qb, :], b2t[:, :]
                    )
                    nc.sync.dma_start(out=out[b * P : (b + 1) * P, :], in_=ob[:])

    if finalize:
        nc.finalize()  # Bacc.compile(): register alloc + ISA codegen + lib loads
    return nc


last_exec_ns = None  # steady-state device wall time of the sharded executable


def _run_pjrt_timed(nc, in_maps, reps=4):
    """run_bass_via_pjrt with the sharded executable re-run and timed.

    Mirrors concourse.bass2jax.run_bass_via_pjrt's multi-core tail; outputs
    are donated zero buffers, so each rep gets fresh zeros. Steady-state
    wall time (min over reps 2..n, includes PJRT dispatch) goes to
    last_exec_ns.
    """
    import time

    import jax
    import jax.numpy as jnp
    from jax.experimental.shard_map import shard_map
    from jax.sharding import Mesh, PartitionSpec

    import concourse.mybir as mybir
    from concourse import bass2jax

    global last_exec_ns
    bass2jax.install_neuronx_cc_hook()
    n_cores = NCORES

    in_names, out_names, out_avals, zero_shapes = [], [], [], []
    for alloc in nc.m.functions[0].allocations:
        if not isinstance(alloc, mybir.MemoryLocationSet):
            continue
        name = alloc.memorylocations[0].name
        if alloc.kind == "ExternalInput":
            in_names.append(name)
        elif alloc.kind == "ExternalOutput":
            np_dt = mybir.dt.np(alloc.dtype)
            out_names.append(name)
            out_avals.append(jax.core.ShapedArray(tuple(alloc.tensor_shape), np_dt))
            zero_shapes.append((tuple(alloc.tensor_shape), np_dt))
    n_params, n_outs = len(in_names), len(out_names)
    all_in_names = tuple(in_names + out_names)

    def _body(*args):
        outs = bass2jax._bass_exec_p.bind(
            *args,
            out_avals=tuple(out_avals),
            in_names=all_in_names,
            out_names=tuple(out_names),
            lowering_input_output_aliases=(),
            sim_require_finite=True,
            sim_require_nnan=True,
            nc=nc,
        )
        return tuple(outs)

    devices = jax.devices()[:n_cores]
    mesh = Mesh(np.asarray(devices), ("core",))
    sharded = jax.jit(
        shard_map(
            _body,
            mesh=mesh,
            in_specs=(PartitionSpec("core"),) * (n_params + n_outs),
            out_specs=(PartitionSpec("core"),) * n_outs,
            check_rep=False,
        ),
        donate_argnums=tuple(range(n_params, n_params + n_outs)),
        keep_unused=True,
    )
    pid_name = nc.partition_id_tensor.name if nc.partition_id_tensor else None

    def _core_input(c, nm):
        if nm == pid_name:
            return np.array([[c]], dtype=np.uint32)
        return np.asarray(in_maps[c][nm])

    concat_in = [
        np.concatenate([_core_input(c, nm) for c in range(n_cores)], axis=0)
        for nm in in_names
    ]
    concat_in = [jax.device_put(a) for a in concat_in]

    def zeros():
        return [
            jnp.zeros((n_cores * s[0], *s[1:]), d) for (s, d) in zero_shapes
        ]

    times = []
    out_arrs = None
    for i in range(reps):
        z = zeros()
        jax.block_until_ready(z)
        t0 = time.perf_counter()
        out_arrs = sharded(*concat_in, *z)
        jax.block_until_ready(out_arrs)
        times.append(time.perf_counter() - t0)
    last_exec_ns = int(min(times[1:]) * 1e9)
    print(f"pjrt call times: {[f'{t * 1e3:.2f}ms' for t in times]}")
    return [
        np.asarray(out_arrs[i]).reshape(n_cores, *out_avals[i].shape)[c]
        for c in range(n_cores)
        for i in [0]
    ]


def measure_hw_exec_ns(consts, in_maps, k=8, reps=10, nc1=None):
    """Amortized per-iteration device time: the kernel body unrolled k times
    in one program vs once, (t_k - t_1)/(k - 1). Cancels the constant
    per-call dispatch overhead of the PJRT/axon path, which dwarfs the
    device time of a single body."""
    if nc1 is None:
        nc1 = _build_program(consts, finalize=True, kloop=1)
    _run_pjrt_timed(nc1, in_maps, reps=reps)
    t1 = last_exec_ns
    nck = _build_program(consts, finalize=True, kloop=k)
    outs = _run_pjrt_timed(nck, in_maps, reps=reps)
    tk = last_exec_ns
    return max((tk - t1) // (k - 1), 1), outs


def kernel(embed, weight, bias1, bias2, edge_src, edge_dst):
    embed = np.asarray(embed)
    weight = np.asarray(weight)
    bias1 = np.asarray(bias1)
    bias2 = np.asarray(bias2)
    edge_src = np.asarray(edge_src)
    edge_dst = np.asarray(edge_dst)

    consts, in_maps = _host_schedules(embed, weight, bias1, bias2, edge_src, edge_dst)

    backend = os.environ.get("KERNEL_BACKEND", "hw")
    if backend == "numpy":
        return _simulate_numpy(consts, in_maps).astype(np.float32)

    nc = _build_program(consts, finalize=backend != "sim")

    if backend == "sim":
        from concourse.bass_interp import MultiCoreSim

        sim = MultiCoreSim(nc, NCORES)
        for k in range(NCORES):
            for name, arr in in_maps[k].items():
                sim.cores[k].tensor(name)[:] = arr
        sim.simulate()
        outs = [np.array(sim.cores[k].tensor("out")) for k in range(NCORES)]
    elif os.environ.get("KERNEL_TRACE", "0") == "1":
        global last_exec_ns
        exec_ns, outs = measure_hw_exec_ns(consts, in_maps, nc1=nc)
        last_exec_ns = exec_ns
    else:
        from concourse.bass_utils import run_bass_kernel_spmd

        res = run_bass_kernel_spmd(nc, in_maps, list(range(NCORES)))
        global last_results
        last_results = res
        outs = [res.results[k]["out"] for k in range(NCORES)]

    full = np.concatenate(outs, 0)[: consts["N"]]
    return np.asarray(full, np.float32)
